# revision 7
# baseline (speedup 1.0000x reference)
"""TRN2 Bass/Tile kernel for nn_BlockSparseMoE (T=2048, D=1024, F=2048, E=8, top-2).

Expert parallelism across the 8 NeuronCores: core c owns expert c. The host
performs routing (top-2 of an [T, E] logit matmul — microseconds of numpy) and
the expert-parallel all-to-all dispatch/combine: it gathers each expert's
tokens into a compact d-major activation block xcT = x[idx_e].T, and after the
device run scatters coef * y back into the full [T, D] output.

The device NEFF is a pure fused GLU FFN per expert, in bf16 (fp32 PSUM
accumulate), sized to the actual max expert load C:

  M12  a = W1 @ xc, b = V1 @ xc  (f-major [128f, C] PSUM chains over 8 d-tiles)
       hT[f] = silu(a) * b  (ACT silu + DVE mult, bf16)
  M3T  yT[d] = sum_f W2[f, d-block]^T-chain @ hT[f]  ([128d, C] PSUM chains
       over 16 f-tiles) — transposed output avoids re-tiling hT and keeps the
       free dim at C; the host transposes yT back during the combine.

Weights are host-swizzled to bf16 so every weight DMA moves contiguous rows,
and all per-rep weight traffic (12 MB) streams behind the ~88 us of PE work.
"""

import os

import numpy as np

import concourse.bass as bass  # noqa: F401  (kept for parity with tooling)
import concourse.mybir as mybir
import concourse.tile as tile
from concourse import bacc
from concourse.bass_utils import run_bass_kernel_spmd

f32 = mybir.dt.float32
bf16 = mybir.dt.bfloat16
AF = mybir.ActivationFunctionType
OP = mybir.AluOpType

np_bf16 = mybir.dt.np(bf16)

_REPS = int(os.environ.get("MOE_REPS", "1"))

P = 128
T = 2048
D = 1024
F = 2048
E = 8
ND = D // P  # 8 d tiles
NF = F // P  # 16 f tiles


def _chunks(C):
    """Split [0, C) into PSUM-bank-sized (<=512) column chunks."""
    out = []
    off = 0
    while off < C:
        w = min(512, C - off)
        out.append((off, w))
        off += w
    return out


def build_moe(C, reps=None):
    global _REPS
    if reps is not None:
        _REPS = reps
    CHS = _chunks(C)

    nc = bacc.Bacc("TRN2", target_bir_lowering=False, debug=False)

    xcT = nc.dram_tensor("xcT", [D, C], bf16, kind="ExternalInput").ap()
    w1s = nc.dram_tensor("w1s", [NF, P, ND * P], bf16, kind="ExternalInput").ap()
    v1s = nc.dram_tensor("v1s", [NF, P, ND * P], bf16, kind="ExternalInput").ap()
    w2s = nc.dram_tensor("w2s", [NF, P, D], bf16, kind="ExternalInput").ap()
    yT = nc.dram_tensor("yT", [D, C], f32, kind="ExternalOutput").ap()

    with tile.TileContext(nc) as tc:
        with (
            tc.tile_pool(name="xct", bufs=2 * ND) as xctpool,
            tc.tile_pool(name="w12", bufs=6) as wpool,
            tc.tile_pool(name="w2p", bufs=NF + 2) as w2pool,
            tc.tile_pool(name="ht", bufs=2 * NF) as htpool,
            tc.tile_pool(name="ssb", bufs=4) as spool,
            tc.tile_pool(name="ysb", bufs=3) as ypool,
            tc.tile_pool(name="psum", bufs=1, space="PSUM") as psp,
        ):
            def _emit_body():
                # compact token activations, d-major: 8 tiles [128d, C]
                xc_sb = [None] * ND
                for d in range(ND):
                    xc_sb[d] = xctpool.tile([P, C], bf16, tag="xct",
                                            name=f"xct_{d}")
                    # ACT hwdge queue: runs in parallel with the w1/v1
                    # weight stream on the SP queue
                    nc.scalar.dma_start(
                        out=xc_sb[d][:], in_=xcT[d * P:(d + 1) * P, :]
                    )

                # ---- M12: hT[f] = silu(W1 xc) * (V1 xc), f-major ----
                hT = [None] * NF
                w2_sb = [None] * NF
                for f in range(NF):
                    hT[f] = htpool.tile([P, C], bf16, tag="ht", name=f"ht_{f}")
                    w1_sb = wpool.tile([P, ND * P], bf16, tag="w12",
                                       name=f"w1_{f}")
                    nc.sync.dma_start(out=w1_sb[:], in_=w1s[f, :, :])
                    v1_sb = wpool.tile([P, ND * P], bf16, tag="w12",
                                       name=f"v1_{f}")
                    nc.sync.dma_start(out=v1_sb[:], in_=v1s[f, :, :])
                    # prefetch this f's W2 tile for M3 while M12 runs
                    # (ACT queue, so it never delays the w1/v1 stream)
                    w2_sb[f] = w2pool.tile([P, D], bf16, tag="w2",
                                           name=f"w2_{f}")
                    nc.scalar.dma_start(out=w2_sb[f][:], in_=w2s[f, :, :])
                    for (off, w) in CHS:
                        a_ps = psp.tile([P, 512], f32, tag="mm", bufs=4,
                                        name="a_ps")
                        for d in range(ND):
                            nc.tensor.matmul(
                                out=a_ps[:, :w],
                                lhsT=w1_sb[:, d * P:(d + 1) * P],
                                rhs=xc_sb[d][:, off:off + w],
                                start=(d == 0), stop=(d == ND - 1),
                            )
                        b_ps = psp.tile([P, 512], f32, tag="mm", bufs=4,
                                        name="b_ps")
                        for d in range(ND):
                            nc.tensor.matmul(
                                out=b_ps[:, :w],
                                lhsT=v1_sb[:, d * P:(d + 1) * P],
                                rhs=xc_sb[d][:, off:off + w],
                                start=(d == 0), stop=(d == ND - 1),
                            )
                        s_sb = spool.tile([P, 512], f32, tag="ssb")
                        nc.scalar.activation(s_sb[:, :w], a_ps[:, :w],
                                             AF.Sigmoid)
                        nc.vector.tensor_tensor(
                            out=s_sb[:, :w], in0=s_sb[:, :w],
                            in1=a_ps[:, :w], op=OP.mult,
                        )
                        nc.vector.tensor_tensor(
                            out=hT[f][:, off:off + w], in0=s_sb[:, :w],
                            in1=b_ps[:, :w], op=OP.mult,
                        )

                # ---- M3T: yT[d] = sum_f w2[f, d-block]^T chains @ hT[f] ----
                for d in range(ND):
                    y_sb = ypool.tile([P, C], f32, tag="ysb", name=f"y_{d}")
                    for (off, w) in CHS:
                        y_ps = psp.tile([P, 512], f32, tag="y", bufs=3,
                                        name="y_ps")
                        for f in range(NF):
                            nc.tensor.matmul(
                                out=y_ps[:, :w],
                                lhsT=w2_sb[f][:, d * P:(d + 1) * P],
                                rhs=hT[f][:, off:off + w],
                                start=(f == 0), stop=(f == NF - 1),
                            )
                        nc.scalar.activation(
                            y_sb[:, off:off + w], y_ps[:, :w], AF.Copy
                        )
                    nc.scalar.dma_start(
                        out=yT[d * P:(d + 1) * P, :], in_=y_sb[:]
                    )

            for _rep in range(_REPS):
                _emit_body()

    return nc


_NC_CACHE = {}


def _get_nc(C, reps=None):
    key = (C, reps if reps is not None else _REPS)
    if key not in _NC_CACHE:
        nc = build_moe(C, reps=reps)
        nc.compile()
        _NC_CACHE[key] = nc
    return _NC_CACHE[key]


def _route(x, gate_w):
    """Host top-2 routing. Returns per-expert (token idx, combine coef)."""
    logits = x.astype(np.float32) @ gate_w.astype(np.float32).T  # [T, E]
    t = np.arange(logits.shape[0])
    sel1 = np.argmax(logits, axis=1)
    l1 = logits[t, sel1]
    masked = logits.copy()
    masked[t, sel1] = -np.inf
    sel2 = np.argmax(masked, axis=1)
    l2 = logits[t, sel2]
    # softmax top-2, L1-renormalized == pairwise sigmoid of the logit gap
    w1c = 1.0 / (1.0 + np.exp(l2 - l1))
    w2c = 1.0 - w1c
    idx, cf = [], []
    for e in range(E):
        m1 = sel1 == e
        m2 = sel2 == e
        ide = np.nonzero(m1 | m2)[0]
        ce = np.where(m1[ide], w1c[ide], w2c[ide]).astype(np.float32)
        idx.append(ide)
        cf.append(ce)
    return idx, cf


def _swizzle_w1(w):
    """(F, D) -> [NF, 128, ND*128] with [f, p, dt*128+fc] = w[f*128+fc, dt*128+p]."""
    v = w.reshape(NF, P, ND, P)  # [f, fc, dt, p]
    return np.ascontiguousarray(v.transpose(0, 3, 2, 1).reshape(NF, P, ND * P))


def _build_in_maps(x, gate_w, w1, v1, w2, C, idx):
    x = np.asarray(x, dtype=np.float32)
    in_maps = []
    for c in range(E):
        ide = idx[c]
        xc = np.zeros((C, D), dtype=np_bf16)
        xc[:len(ide)] = x[ide].astype(np_bf16)
        in_maps.append({
            "xcT": np.ascontiguousarray(xc.T),
            "w1s": _swizzle_w1(np.asarray(w1[c], np.float32).astype(np_bf16)),
            "v1s": _swizzle_w1(np.asarray(v1[c], np.float32).astype(np_bf16)),
            "w2s": np.ascontiguousarray(
                np.asarray(w2[c], np.float32).astype(np_bf16).reshape(NF, P, D)
            ),
        })
    return in_maps


def _capacity(idx):
    C = max(len(i) for i in idx)
    return max(16, (C + 3) // 4 * 4)  # 4-align DMA rows


def kernel(x, gate_w, w1, v1, w2):
    idx, cf = _route(x, gate_w)
    C = _capacity(idx)
    nc = _get_nc(C)
    in_maps = _build_in_maps(x, gate_w, w1, v1, w2, C, idx)
    res = run_bass_kernel_spmd(nc, in_maps, core_ids=list(range(E)))
    out = np.zeros((T, D), dtype=np.float32)
    for c, r in enumerate(res.results):
        n = len(idx[c])
        y = r["yT"].T[:n]  # [n, D] unscaled expert output
        out[idx[c]] += cf[c][:, None] * y
    return out
